# revision 65
# baseline (speedup 1.0000x reference)
"""Multi-head attention (B=4, S=2048, D=1024, H=16) on 8 Trainium2 cores.

Sharding: core c handles batch b=c//2 and head-group g=c%2 (8 heads, 512 of
the 1024 head dims).  Each core computes q/k/v projections for its head
slice, causal attention for its 8 heads, and a partial output projection
(contraction over its 512 concat dims).  The host sums the two partials per
batch and adds the dense bias.  No on-device collectives needed.

Per-core kernel (fp32 accumulation throughout):
  - Host pre-arranges all inputs into p-major layouts so every device DMA
    is contiguous per partition; q/k inputs and weights ship as fp8-e4m3
    (weights x32 so they stay normal-range, undone in the PSUM->SBUF copy).
  - q/k projections run fp8 DoubleRow matmuls (2 kt-subtiles per
    instruction, 2x ALU rate); v projection and dense stay bf16.
  - Startup loads are issued in need order (k, q, v) so the first matmul
    and each following projection start as soon as their bytes land.
  - Per 512-row sq-chunk: k proj, q proj, v proj, then attention for the
    8 heads in pairs, software-pipelined: QK(j+1) is emitted before AV(j)
    so the PE's in-order queue never stalls on ScalarE's exp(j).  The QK
    pair row-tiles the array (head h in rows 0-63, h+1 in 64-127) and the
    previous chunk's dense matmuls interleave as PE filler.
  - AV accumulates outT_aug [65, sq] in PSUM (row 64 = exp row sums).
    Normalization: copy both heads' av to SBUF, DMA-roundtrip broadcast of
    the sums, reciprocal_approx_fast on DVE, multiply into concatT.
  - Last chunk: dense kt 0-2 runs as filler inside the final pair's
    attention (partials to SBUF); the final pair normalizes straight out
    of PSUM with a rank-1 PE broadcast matmul, so only the kt=3 dense pass
    plus an add trails the last AV on the critical tail.
"""

import ml_dtypes
import numpy as np

import concourse.bass as bass
from concourse import bacc
import concourse.mybir as mybir
import concourse.tile as tile
from concourse.bass_utils import run_bass_kernel_spmd
from concourse.masks import make_identity, make_upper_triangular

B, S, D, H = 4, 2048, 1024, 16
DEPTH = 64
HPC = 8          # heads per core
DH = HPC * DEPTH  # 512: per-core head width
N_CORES = 8
SCALE = 1.0 / 32.0  # 1/sqrt(D)
NEG = -1e9 * 32.0   # mask bias, pre-divided by SCALE

FR = mybir.dt.float32r
F32 = mybir.dt.float32
BF = mybir.dt.bfloat16
F8 = mybir.dt.float8e4
FP8_QK = True     # store x/w for the q,k projections in fp8 (DoubleRow)
W8SC = 32.0       # host-side weight rescale so fp8 weights stay normal-range

Exp = mybir.ActivationFunctionType.Exp
Copy = mybir.ActivationFunctionType.Copy
Ident = mybir.ActivationFunctionType.Identity

NT = S // 128    # 16 sequence tiles of 128
NCH = S // 512   # 4 sequence chunks of 512
KT = D // 128    # 8 contraction tiles for the projections
CH = 512         # row-chunk for the input transpose pipeline


def _build(variant: str, with_bias: bool) -> bass.Bass:
    """variant: 'causal' (triu mask), 'full' (no mask), 'general' (additive)."""
    nc = bacc.Bacc()

    # host pre-arranged p-major layouts (contiguous per-partition DMAs)
    QKD = F8 if FP8_QK else BF
    xqT = nc.declare_dram_parameter("xqT", [NCH, 128, KT, CH], QKD,
                                    isOutput=False).ap()
    xkT = nc.declare_dram_parameter("xkT", [NCH, 128, KT, CH], QKD,
                                    isOutput=False).ap()
    xvT = nc.declare_dram_parameter("xvT", [NCH, 128, KT, CH], BF,
                                    isOutput=False).ap()
    wqT = nc.declare_dram_parameter("wqT", [128, KT, DH], QKD,
                                    isOutput=False).ap()
    wkT = nc.declare_dram_parameter("wkT", [128, KT, DH], QKD,
                                    isOutput=False).ap()
    wvT = nc.declare_dram_parameter("wvT", [128, KT, DH], BF,
                                    isOutput=False).ap()
    dwT = nc.declare_dram_parameter("dwT", [128, DH // 128, D], BF,
                                    isOutput=False).ap()
    if with_bias:
        qb = nc.declare_dram_parameter("qb", [DH], F32, isOutput=False).ap()
        kb = nc.declare_dram_parameter("kb", [DH], F32, isOutput=False).ap()
        vb = nc.declare_dram_parameter("vb", [DH], F32, isOutput=False).ap()
    if variant == "general":
        # mask.T pre-scaled by -1e9/SCALE, [sk, sq]
        mT = nc.declare_dram_parameter("mT", [S, S], F32, isOutput=False).ap()
    outp = nc.declare_dram_parameter("outp", [S, D], F32, isOutput=True).ap()

    with tile.TileContext(nc) as tc:
        with (
            tc.tile_pool(name="const", bufs=1) as const,
            tc.tile_pool(name="wpool", bufs=1) as wpool,

            tc.tile_pool(name="xp", bufs=3) as x_pool,
            tc.tile_pool(name="ptp", bufs=6) as pt_pool,
            tc.tile_pool(name="nrm", bufs=4) as nrm_pool,
            tc.tile_pool(name="mskp", bufs=2) as msk_pool,
            tc.tile_pool(name="otp", bufs=3) as ot_pool,
            # PSUM budget (8 banks): p=2, lg=2x2banks, av=2
            tc.tile_pool(name="pjs", bufs=2, space="PSUM") as p_psum,
            tc.tile_pool(name="lgs", bufs=2, space="PSUM") as lg_psum,
            tc.tile_pool(name="avs", bufs=2, space="PSUM") as av_psum,
            tc.tile_pool(name="drs", bufs=4, space="DRAM") as dr_pool,
        ):
            if variant == "causal":
                binm = const.tile([128, 128], BF)
                # binm[r, q] = 1 if q >= r else 0 (keep sq >= sk in diag block)
                make_upper_triangular(nc, binm, val=1.0, diag=True)
            # row of ones at partition 64 (matches the av2 sums row) for the
            # rank-1 tail broadcast matmul
            ones1f = const.tile([65, 64], F32)
            nc.vector.memset(ones1f[64:65, :], 1.0)
            ones1p = const.tile([65, 64], FR)
            nc.vector.tensor_copy(ones1p[64:65, :], ones1f[64:65, :])
            ones1 = ones1p[64:65, :]
            qhT = [const.tile([128, DH // 128, CH], BF, name=f"qhT{i}")
                   for i in range(NCH)]
            khT = const.tile([128, DH // 128, S], BF)
            vha = const.tile([128, NT, HPC, DEPTH + 1], BF)
            nc.vector.memset(vha[:, :, :, DEPTH], 1.0)
            catT = const.tile([128, DH // 128, S], BF)

            if with_bias:
                qb_sb = const.tile([128, DH // 128], F32)
                nc.sync.dma_start(out=qb_sb,
                                  in_=qb.rearrange("(m p) -> p m", p=128))
                kb_sb = const.tile([128, DH // 128], F32)
                nc.sync.dma_start(out=kb_sb,
                                  in_=kb.rearrange("(m p) -> p m", p=128))
                vb_bc = const.tile([128, HPC, DEPTH], F32)
                nc.sync.dma_start(
                    out=vb_bc,
                    in_=vb.rearrange("(h d) -> h d", h=HPC)
                    .unsqueeze(0)
                    .partition_broadcast(128),
                )

            def load_x(xT_dram, c, nm, split=1, eng=None, dt=BF):
                """one CH-chunk of a transposed input -> [128, KT, CH]"""
                xs = x_pool.tile([128, KT, CH], dt, tag=f"x{dt}", name=nm)
                step = KT // split
                for i in range(split):
                    (eng or nc.sync).dma_start(
                        out=xs[:, i * step : (i + 1) * step],
                        in_=xT_dram[c, :, i * step : (i + 1) * step])
                return xs

            def proj_T(x_sb, wt, dst, bsb, c, hooks=None):
                """qhT/khT-style projection of one CH-chunk into dst.
                fp8 inputs use DoubleRow (2 kt-subtiles per matmul, 2x ALU)
                with the host's x{W8SC} weight scale undone in the copy."""
                for _ in proj_steps(x_sb, wt, dst, bsb, c, hooks):
                    pass

            def proj_steps(x_sb, wt, dst, bsb, c, hooks=None):
                """proj_T as a generator (one matmul/copy per yield) so it
                can interleave into attention j-loops as PE filler."""
                dr = wt.dtype == F8
                osc = (1.0 / W8SC) if dr else 1.0
                for m in range(DH // 128):
                    ps = p_psum.tile([128, CH], F32, tag="pj")
                    if dr:
                        for k2 in range(KT // 2):
                            nc.tensor.matmul(
                                ps,
                                lhsT=wt[:, 2 * k2 : 2 * k2 + 2,
                                        128 * m : 128 * (m + 1)],
                                rhs=x_sb[:, 2 * k2 : 2 * k2 + 2, :],
                                start=(k2 == 0),
                                stop=(k2 == KT // 2 - 1),
                                perf_mode=mybir.MatmulPerfMode.DoubleRow,
                            )
                            yield
                    else:
                        for kt in range(KT):
                            nc.tensor.matmul(
                                ps,
                                lhsT=wt[:, kt, 128 * m : 128 * (m + 1)],
                                rhs=x_sb[:, kt, :],
                                start=(kt == 0),
                                stop=(kt == KT - 1),
                            )
                            yield
                    out_ap = (dst[c][:, m, :] if isinstance(dst, list)
                              else dst[:, m, c * CH : (c + 1) * CH])
                    if with_bias:
                        nc.scalar.activation(out_ap, ps, Ident, scale=osc,
                                             bias=bsb[:, m : m + 1])
                    else:
                        # DVE copy keeps ScalarE (the exp engine) unloaded
                        nc.vector.tensor_scalar_mul(out_ap, ps, osc)
                    if hooks and m in hooks:
                        hooks[m]()
                    yield

            def proj_v(x_sb, wt, c):
                """v projection of one CH-chunk into vha (natural layout)."""
                for t in range(CH // 128):
                    j = (c * CH) // 128 + t
                    ps = p_psum.tile([128, DH], F32, tag="pj")
                    for kt in range(KT):
                        nc.tensor.matmul(
                            ps,
                            lhsT=x_sb[:, kt, 128 * t : 128 * (t + 1)],
                            rhs=wt[:, kt, :],
                            start=(kt == 0),
                            stop=(kt == KT - 1),
                        )
                    psv = ps.rearrange("p (h d) -> p h d", h=HPC)
                    if with_bias:
                        nc.vector.tensor_add(vha[:, j, :, 0:DEPTH], psv, vb_bc)
                    else:
                        # ScalarE is idle during projection phases; keep the
                        # DVE free for the previous chunk's normalization
                        nc.scalar.copy(vha[:, j, :, 0:DEPTH], psv)

            def attention_pair(h0, c, filler=None, rate=0.0, skip_j=0,
                               tail=False):
                """Two heads (h0, h0+1) interleaved per j-block so the PE
                never stalls on ScalarE exp, and the odd head's lhsT sits in
                row groups 2-3 (LDWEIGHTS overlap with the even head).
                After each j-block, ~rate steps of `filler` (independent PE
                work) are emitted to keep the PE dense while ScalarE runs.
                skip_j delays filler onset (e.g. when it depends on the
                previous pair's normalization finishing)."""
                heads = (h0, h0 + 1)
                fill_acc = 0.0
                jmax = 4 * c + 3 if variant == "causal" else NT - 1
                avs = {}
                for h in heads:
                    avs[h] = av_psum.tile([65, 512], F32, tag="av",
                                          name=f"av_{h}_{c}")
                def emit_qk_exp(j):
                    """QK pair + exp for block j; returns pts handles."""
                    t = j - 4 * c
                    off = 128 * t if (variant == "causal" and t >= 0) else 0
                    mblk = None
                    if variant == "general":
                        mblk = msk_pool.tile([128, 512], F32, tag="mb")
                        nc.sync.dma_start(
                            out=mblk,
                            in_=mT[128 * j : 128 * (j + 1),
                                   512 * c : 512 * (c + 1)],
                        )
                    lg2 = lg_psum.tile([128, 1024], F32, tag="lg",
                                       name=f"lg2_{c}_{j}")
                    pt2 = pt_pool.tile([128, 1024], BF, tag="pt",
                                       name=f"pt2_{c}_{j}")
                    pts = {}
                    for i, h in enumerate(heads):
                        p0 = 64 * (h % 2)
                        lo = 512 * i
                        nc.tensor.matmul(
                            lg2[:, lo + off : lo + 512],
                            lhsT=khT[p0 : p0 + 64, h // 2,
                                     128 * j : 128 * (j + 1)],
                            rhs=qhT[c][p0 : p0 + 64, h // 2, off:],
                            start=True,
                            stop=True,
                        )
                        if mblk is not None:
                            nc.vector.tensor_add(lg2[:, lo : lo + 512],
                                                 lg2[:, lo : lo + 512], mblk)
                        pts[h] = pt2[:, lo : lo + 512]
                    if variant == "causal" and t >= 0:
                        # diagonal block: exp the two valid halves separately
                        # and apply the triangular mask
                        for i, h in enumerate(heads):
                            lo = 512 * i
                            nc.scalar.activation(
                                pt2[:, lo + off : lo + 512],
                                lg2[:, lo + off : lo + 512], Exp, scale=SCALE)
                            nc.gpsimd.tensor_mul(
                                pt2[:, lo + off : lo + off + 128],
                                pt2[:, lo + off : lo + off + 128], binm)
                    else:
                        nc.scalar.activation(pt2, lg2, Exp, scale=SCALE)
                    return pts, off

                def emit_av(j, pts, off):
                    for h in heads:
                        nc.tensor.matmul(
                            avs[h][:, off:],
                            lhsT=vha[:, j, h, :],
                            rhs=pts[h][:, off:] if off else pts[h],
                            start=(j == 0),
                            stop=(j == jmax),
                        )

                # software pipeline: QK(j+1) is emitted before AV(j), so the
                # PE's in-order queue never stalls waiting on exp(j)
                prev = None
                for j in range(jmax + 1):
                    state = emit_qk_exp(j)
                    if prev is not None:
                        if filler is not None and j - 1 >= skip_j:
                            fill_acc += rate
                            while fill_acc >= 1.0:
                                fill_acc -= 1.0
                                if next(filler, None) is StopIteration:
                                    break
                        emit_av(j - 1, *prev)
                    prev = state
                if filler is not None:
                    fill_acc += rate
                    while fill_acc >= 1.0:
                        fill_acc -= 1.0
                        if next(filler, None) is StopIteration:
                            break
                emit_av(jmax, *prev)
                # normalize by exp row sums (row 64 of av). Copy both heads'
                # av out of PSUM fast (releases the banks), then one PE-free
                # DRAM-roundtrip broadcast of the pair's sums, one fast
                # approximate reciprocal, multiply into concatT.
                mt = h0 // 2
                rb = nrm_pool.tile([64, 1024], F32, tag="rb")
                if tail:
                    # final pair: normalize straight out of PSUM, broadcast
                    # the sums via a rank-1 PE matmul (PE idles here) instead
                    # of the DRAM roundtrip — keeps the last normalization
                    # off the kernel's critical tail
                    # head h0+1 is processed first throughout: its half needs
                    # the extra partition-shift DMA into catT, which gates
                    # the trailing dense pass
                    rs_fr = nrm_pool.tile([65, 1024], FR, tag="rsf")
                    rb_ps = lg_psum.tile([64, 1024], F32, tag="lg",
                                         name=f"rbps_{c}")
                    for i in (1, 0):
                        nc.scalar.copy(rs_fr[64:65, 512 * i : 512 * (i + 1)],
                                       avs[heads[i]][64:65, :])
                    for i in (1, 0):
                        nc.tensor.matmul(
                            rb_ps[:, 512 * i : 512 * (i + 1)], lhsT=ones1,
                            rhs=rs_fr[64:65, 512 * i : 512 * (i + 1)],
                            start=True, stop=True)
                        nc.vector.reciprocal_approx_fast(
                            rb[:, 512 * i : 512 * (i + 1)],
                            rb_ps[:, 512 * i : 512 * (i + 1)])
                    a0 = avs[heads[0]][0:64, :]
                    a1 = avs[heads[1]][0:64, :]
                else:
                    av2 = nrm_pool.tile([65, 1024], F32, tag="av2",
                                        name=f"av2_{h0}_{c}")
                    # one copy per engine so both run concurrently and the
                    # row-sum DMA can start a copy-length earlier
                    nc.vector.tensor_copy(av2[:, 0:512], avs[heads[0]])
                    nc.scalar.copy(av2[:, 512:1024], avs[heads[1]])
                    rs_dr = dr_pool.tile([1024], F32, tag="rsd")
                    nc.sync.dma_start(out=rs_dr, in_=av2[64:65, :])
                    nc.sync.dma_start(
                        out=rb, in_=rs_dr.unsqueeze(0).partition_broadcast(64))
                    nc.vector.reciprocal_approx_fast(rb, rb)
                    a0 = av2[0:64, 0:512]
                    a1 = av2[0:64, 512:1024]
                bnc = nrm_pool.tile([64, 512], BF, tag="bnc")
                nc.vector.tensor_mul(bnc, a1, rb[:, 512:1024])
                nc.sync.dma_start(
                    out=catT[64:128, mt, 512 * c : 512 * (c + 1)], in_=bnc)
                nc.vector.tensor_mul(
                    catT[0:64, mt, 512 * c : 512 * (c + 1)], a0, rb[:, 0:512])

            def dense_steps(wt, c, kts=None, accum=False):
                """partial output projection for sq-chunk c, one matmul per
                yield — interleaved into attention j-loops as PE filler.
                kts: subset of contraction tiles (split passes; the second
                pass accumulates into DRAM via the DMA compute engine)."""
                if kts is None:
                    kts = list(range(DH // 128))
                for st in range(4 * c, 4 * (c + 1)):
                    for oc in range(D // 512):
                        ps = p_psum.tile([128, 512], F32, tag="pj",
                                         name=f"dps_{st}_{oc}")
                        for i, kt in enumerate(kts):
                            nc.tensor.matmul(
                                ps,
                                lhsT=catT[:, kt, 128 * st : 128 * (st + 1)],
                                rhs=wt[:, kt, 512 * oc : 512 * (oc + 1)],
                                start=(i == 0),
                                stop=(i == len(kts) - 1),
                            )
                            yield
                        ob = ot_pool.tile([128, 512], F32, tag="ob",
                                          name=f"ob_{st}_{oc}")
                        nc.vector.tensor_copy(ob, ps)
                        if accum:
                            nc.gpsimd.dma_start(
                                out=outp[128 * st : 128 * (st + 1),
                                         512 * oc : 512 * (oc + 1)],
                                in_=ob, accum_op=mybir.AluOpType.add)
                        else:
                            nc.sync.dma_start(
                                out=outp[128 * st : 128 * (st + 1),
                                         512 * oc : 512 * (oc + 1)],
                                in_=ob)
                        yield

            def dense(wt, c):
                for _ in dense_steps(wt, c):
                    pass

            ob1_all = const.tile([128, 4 * (D // 512), 512], BF)

            def dense_pass1(wt, c):
                """last chunk's dense, kt 0-2 only (heads 0-5): runs as
                filler inside the final pair's attention, partials to SBUF"""
                idx = 0
                for st in range(4 * c, 4 * (c + 1)):
                    for oc in range(D // 512):
                        ps = p_psum.tile([128, 512], F32, tag="pj",
                                         name=f"dp1_{st}_{oc}")
                        for i, kt in enumerate([0, 1, 2]):
                            nc.tensor.matmul(
                                ps,
                                lhsT=catT[:, kt, 128 * st : 128 * (st + 1)],
                                rhs=wt[:, kt, 512 * oc : 512 * (oc + 1)],
                                start=(i == 0),
                                stop=(i == 2),
                            )
                            yield
                        nc.vector.tensor_copy(ob1_all[:, idx % 8, :], ps)
                        idx += 1
                        yield

            def dense_pass2(wt, c):
                """kt=3 (heads 6-7) + SBUF partial: the only dense work that
                trails the final pair's normalization"""
                idx = 0
                for st in range(4 * c, 4 * (c + 1)):
                    for oc in range(D // 512):
                        ps = p_psum.tile([128, 512], F32, tag="pj",
                                         name=f"dp2_{st}_{oc}")
                        nc.tensor.matmul(
                            ps,
                            lhsT=catT[:, 3, 128 * st : 128 * (st + 1)],
                            rhs=wt[:, 3, 512 * oc : 512 * (oc + 1)],
                            start=True, stop=True,
                        )
                        ob = ot_pool.tile([128, 512], F32, tag="ob",
                                          name=f"ob2_{st}_{oc}")
                        nc.vector.tensor_add(ob, ps, ob1_all[:, idx % 8, :])
                        idx += 1
                        nc.gpsimd.dma_start(
                            out=outp[128 * st : 128 * (st + 1),
                                     512 * oc : 512 * (oc + 1)],
                            in_=ob)

            # ---- k and v projections ----
            # startup is HBM-bound: only xk0+wk are allowed to transfer
            # first; every later chunk-0 load is gated behind a k-proj copy
            # so it can't steal bandwidth from the critical first loads
            # fp8 halves the critical xk0/wk bytes, so the (bf16) v loads
            # start immediately too; the fp8 q loads are gated behind the
            # first k-proj copy
            xk0 = load_x(xkT, 0, "xk0pre", split=2, dt=QKD)
            wk_sb = wpool.tile([128, KT, DH], QKD, tag="wk")
            nc.scalar.dma_start(out=wk_sb[:, 0 : KT // 2], in_=wkT[:, 0 : KT // 2])
            nc.scalar.dma_start(out=wk_sb[:, KT // 2 :], in_=wkT[:, KT // 2 :])
            wq_sb = wpool.tile([128, KT, DH], QKD, tag="wq")
            nc.scalar.dma_start(out=wq_sb, in_=wqT)
            wv_sb = wpool.tile([128, KT, DH], BF, tag="wv")
            nc.gpsimd.dma_start(out=wv_sb, in_=wvT)
            wd_sb = wpool.tile([128, DH // 128, D], BF, tag="wd")
            x0_held = {"q": load_x(xqT, 0, "xq0", split=2, dt=QKD),
                       "v": load_x(xvT, 0, "xv0")}
            w_hooks = {}
            wd_hook = {0: lambda: nc.gpsimd.dma_start(out=wd_sb, in_=dwT)}

            if variant == "causal":
                # causal: attention chunk c only needs k/v rows < 512(c+1),
                # so k/v/q projections interleave with attention per chunk.
                # q (small fp8 loads) runs before v so the larger bf16 v
                # transfers have more compute to hide behind at startup
                for cq in range(NCH):
                    if cq == 0:
                        proj_T(xk0, wk_sb, khT,
                               kb_sb if with_bias else None, 0, hooks=w_hooks)
                    elif cq > 1:
                        xs = load_x(xkT, cq, f"xk{cq}", eng=nc.gpsimd,
                                    dt=QKD)
                        proj_T(xs, wk_sb, khT,
                               kb_sb if with_bias else None, cq)
                    xs = (x0_held["q"] if cq == 0
                          else load_x(xqT, cq, f"xq{cq}", eng=nc.gpsimd,
                                      dt=QKD))
                    proj_T(xs, wq_sb, qhT, qb_sb if with_bias else None, cq,
                           hooks=wd_hook if cq == 0 else None)
                    xs = (x0_held["v"] if cq == 0
                          else load_x(xvT, cq, f"xv{cq}", eng=nc.gpsimd))
                    proj_v(xs, wv_sb, cq)
                    nsteps = 4 * (DH // 128 + 1) * (D // 512)
                    if cq == 0:
                        # chunk 0 has no previous dense to fill its attention
                        # with — use chunk 1's k-projection instead (reads
                        # xk1, writes khT columns attention(0) never touches)
                        xk1s = load_x(xkT, 1, "xk1", dt=QKD)
                        fK = proj_steps(xk1s, wk_sb, khT,
                                        kb_sb if with_bias else None, 1)
                        nK = (KT // 2 if FP8_QK else KT + 1) * (DH // 128)
                        attention_pair(0, cq)
                        for h0 in range(2, HPC, 2):
                            attention_pair(h0, cq, fK, nK / 12 + 0.05)
                        for _ in fK:
                            pass
                    elif cq < NCH - 1:
                        filler = dense_steps(wd_sb, cq - 1)
                        njs = (HPC // 2) * (4 * cq + 4)
                        rate = nsteps / njs + 0.01
                        for h0 in range(0, HPC, 2):
                            attention_pair(h0, cq, filler, rate)
                        for _ in filler:  # drain any remainder
                            pass
                    else:
                        # last chunk: previous chunk's dense fills pairs 0-2;
                        # this chunk's dense (kt 0-2) fills the final pair so
                        # only dense_pass2 trails the last normalization
                        njp = 4 * cq + 4
                        fA = dense_steps(wd_sb, cq - 1)
                        rA = nsteps / (3 * njp) + 0.01
                        for h0 in (0, 2, 4):
                            attention_pair(h0, cq, fA, rA)
                        for _ in fA:
                            pass
                        fB = dense_pass1(wd_sb, cq)
                        attention_pair(HPC - 2, cq, fB,
                                       32 / (njp - 3) + 0.05, skip_j=3,
                                       tail=True)
                        for _ in fB:
                            pass
                        dense_pass2(wd_sb, cq)
            else:
                # full/general: every attention chunk reads all of k/v
                for cb in w_hooks.values():
                    cb()
                for cb in wd_hook.values():
                    cb()
                for cq in range(NCH):
                    xs = xk0 if cq == 0 else load_x(xkT, cq, f"xk{cq}", dt=QKD)
                    proj_T(xs, wk_sb, khT, kb_sb if with_bias else None, cq)
                    xs = (x0_held["v"] if cq == 0
                          else load_x(xvT, cq, f"xv{cq}"))
                    proj_v(xs, wv_sb, cq)
                for cq in range(NCH):
                    xs = (x0_held["q"] if cq == 0
                          else load_x(xqT, cq, f"xq{cq}", dt=QKD))
                    proj_T(xs, wq_sb, qhT, qb_sb if with_bias else None, cq)
                    for h0 in range(0, HPC, 2):
                        attention_pair(h0, cq)
                    dense(wd_sb, cq)

    nc.compile()
    return nc


_CACHE: dict = {}


def _get_nc(variant: str, with_bias: bool) -> bass.Bass:
    key = (variant, with_bias)
    if key not in _CACHE:
        _CACHE[key] = _build(variant, with_bias)
    return _CACHE[key]


def _pmaj_x(xT):
    """[D, S] -> [NCH, 128, KT, CH] p-major chunked layout."""
    return np.ascontiguousarray(
        xT.reshape(KT, 128, NCH, CH).transpose(2, 1, 0, 3))


def _pmaj_w(wT):
    """[D, DH] -> [128, KT, DH] p-major layout."""
    return np.ascontiguousarray(wT.reshape(KT, 128, DH).transpose(1, 0, 2))


def _make_in_maps(q, k, v, mask2d, wq_w, wk_w, wv_w, dense_w,
                  wq_b, wk_b, wv_b, variant, with_bias):
    qk_dt = ml_dtypes.float8_e4m3fn if FP8_QK else ml_dtypes.bfloat16
    xT_cache = {}
    for b in range(B):
        xT_cache[b] = tuple(
            _pmaj_x(x[b].T.astype(dt))
            for x, dt in ((q, qk_dt), (k, qk_dt), (v, ml_dtypes.bfloat16)))
    wsc = np.float32(W8SC) if FP8_QK else np.float32(1.0)
    in_maps = []
    for core in range(N_CORES):
        b, g = divmod(core, 2)
        rows = slice(DH * g, DH * (g + 1))
        im = {
            "xqT": xT_cache[b][0],
            "xkT": xT_cache[b][1],
            "xvT": xT_cache[b][2],
            "wqT": _pmaj_w((wq_w[rows].T * wsc).astype(qk_dt)),
            "wkT": _pmaj_w((wk_w[rows].T * wsc).astype(qk_dt)),
            "wvT": _pmaj_w(wv_w[rows].T.astype(ml_dtypes.bfloat16)),
            "dwT": np.ascontiguousarray(
                dense_w[:, rows].T.astype(ml_dtypes.bfloat16)
                .reshape(DH // 128, 128, D).transpose(1, 0, 2)),
        }
        if with_bias:
            im["qb"] = np.ascontiguousarray(wq_b[rows])
            im["kb"] = np.ascontiguousarray(wk_b[rows])
            im["vb"] = np.ascontiguousarray(wv_b[rows])
        if variant == "general":
            im["mT"] = np.ascontiguousarray(mask2d.T * np.float32(NEG))
        in_maps.append(im)
    return in_maps


def kernel(q, k, v, mask, wq_w, wq_b, wk_w, wk_b, wv_w, wv_b,
           dense_w, dense_b, **run_kwargs):
    q = np.asarray(q, np.float32)
    k = np.asarray(k, np.float32)
    v = np.asarray(v, np.float32)
    mask2d = np.asarray(mask, np.float32).reshape(S, S)
    wq_w = np.asarray(wq_w, np.float32)
    wk_w = np.asarray(wk_w, np.float32)
    wv_w = np.asarray(wv_w, np.float32)
    dense_w = np.asarray(dense_w, np.float32)
    wq_b = np.asarray(wq_b, np.float32)
    wk_b = np.asarray(wk_b, np.float32)
    wv_b = np.asarray(wv_b, np.float32)
    dense_b = np.asarray(dense_b, np.float32)

    causal_ref = np.triu(np.ones((S, S), np.float32), k=1)
    if np.array_equal(mask2d, causal_ref):
        variant = "causal"
    elif not mask2d.any():
        variant = "full"
    else:
        variant = "general"
    with_bias = bool(wq_b.any() or wk_b.any() or wv_b.any())

    nc = _get_nc(variant, with_bias)
    in_maps = _make_in_maps(q, k, v, mask2d, wq_w, wk_w, wv_w, dense_w,
                            wq_b, wk_b, wv_b, variant, with_bias)
    res = run_bass_kernel_spmd(nc, in_maps, core_ids=list(range(N_CORES)),
                               **run_kwargs)
    outs = res.results
    out = np.empty((B, S, D), np.float32)
    for b in range(B):
        out[b] = outs[2 * b]["outp"] + outs[2 * b + 1]["outp"]
    out += dense_b[None, None, :].astype(np.float32)
    globals()["_last_results"] = res
    return out

